# revision 1
# baseline (speedup 1.0000x reference)
"""Cumulative-probability head on 8 Trainium2 NeuronCores.

out[b, j] = sum_{i<=j} relu(x @ W_h^T + b_h)[b, i] + (x @ W_base^T + b_base)[b]

Data-parallel: x is sharded along batch (1024 rows per core); the small
weights are replicated. The host passes x pre-transposed per shard
([D, 1024], contiguous rows) so the contraction dim lands on SBUF
partitions with no on-device transposes. Per core:

  1. haz = xT.T @ WT_aug in float32r (FP22 multiplies, fp32 PSUM accum).
     WT_aug is [2049, 516]: hazard cols 0..511, base col 512, 3 zero pad
     cols; row 2048 is the bias row, added post-accumulation on DVE
     via a partition-broadcast read. The 516 output cols split into two
     even N=258 PSUM banks (fp32r requires an even moving dim).
  2. Each 128-row batch tile accumulates the full K=2048 contraction
     in a pair of PSUM banks; four tiles are in flight (8 banks), so
     the PE has work while input chunks stream in.
  3. Input DMAs are spread over three queue rings (Sync HWDGE, Scalar
     HWDGE, GPSIMD SWDGE) in k-order — one ring caps at ~160 GB/s,
     which would serialize the 12 MB of per-core input.
  4. ReLU on ScalarE (hazard cols only; base col stays unactivated),
     then the inclusive cumsum along T on DVE via tensor_tensor_scan
     with the base hazard as the per-partition initial state.
"""

import numpy as np

import concourse.bass as bass
import concourse.tile as tile
from concourse import bacc, mybir
from concourse.bass_utils import run_bass_kernel_spmd

B, D, T = 8192, 2048, 512
NCORES = 8
BLOC = B // NCORES            # 1024 rows per core
NB = BLOC // 128              # 8 batch tiles per core
NK = D // 128                 # 16 contraction chunks
TP = 516                      # padded output width (512 hazard + base + 3 junk)
NA = 258                      # output cols 0..257 in PSUM bank A
NBK = TP - NA                 # cols 258..515 in bank B (col 512 = base)
BOFF = T - NA                 # offset of the base col inside bank B (254)

F32 = mybir.dt.float32
F32R = mybir.dt.float32r


def _build_program():
    nc = bacc.Bacc("TRN2", target_bir_lowering=False, debug=False)

    xt_d = nc.dram_tensor("xt", [D, BLOC], F32R, kind="ExternalInput")
    wt_d = nc.dram_tensor("wt", [D + 1, TP], F32R, kind="ExternalInput")
    out_d = nc.dram_tensor("out", [BLOC, T], F32, kind="ExternalOutput")

    with tile.TileContext(nc) as tc:
        with (
            tc.tile_pool(name="consts", bufs=1) as consts,
            tc.tile_pool(name="wt", bufs=1) as wtp,
            tc.tile_pool(name="xt", bufs=1) as xtp,
            tc.tile_pool(name="haz", bufs=4) as hazp,
            tc.tile_pool(name="outp", bufs=4) as outp,
            tc.tile_pool(name="ps_mm", bufs=4, space="PSUM") as ps_mm,
        ):
            zeros = consts.tile([128, T], F32)
            nc.vector.memset(zeros, 0.0)

            # Input loads, k-ordered, spread over three DMA rings. The two
            # HWDGE rings (Sync, Scalar) are faster than the GPSIMD SWDGE
            # ring, so the first chunks — which gate the PE pipeline ramp —
            # go to the HWDGE rings, and the slow ring only carries late
            # chunks. Weights (half the size of an x chunk) ride opposite
            # rings from their x chunk so the pair lands together.
            XT_RING = [0, 1, 0, 1, 2, 0, 1, 2, 0, 1, 2, 0, 1, 2, 0, 1]
            WT_RING = [1, 0, 1, 0, 0, 1, 2, 0, 1, 2, 0, 1, 2, 0, 1, 2]
            rings = [nc.sync, nc.scalar, nc.gpsimd]
            xt_tiles = []
            wt_tiles = []
            wbias_bc = wtp.tile([128, TP], F32, tag="wbias")
            H = BLOC // 2
            for k in range(NK):
                # x chunks load in column halves: the first four (live)
                # batch tiles only read cols 0..511, so streaming ALL
                # first-halves before any second-half lets them retire on
                # half the input volume and frees PSUM banks mid-window
                # for batch tiles 4..7.
                xk = xtp.tile([128, BLOC], F32R, tag=f"xt{k}")
                rings[XT_RING[k]].dma_start(
                    out=xk[:, 0:H], in_=xt_d[128 * k : 128 * (k + 1), 0:H]
                )
                xt_tiles.append(xk)
                w = wtp.tile([128, TP], F32R, tag=f"wt{k}")
                rings[WT_RING[k]].dma_start(out=w, in_=wt_d[128 * k : 128 * (k + 1), :])
                wt_tiles.append(w)
                if k == 8:
                    # Bias row replicated across all 128 partitions with a
                    # partition-stride-0 DMA read (engines can't read
                    # stride-0 partition APs, but DMA can). Emitted mid-
                    # stream on the GPSIMD ring: early enough to be resident
                    # long before the first b-tile's bias add (which gates
                    # PSUM bank recycling), late enough not to delay the
                    # chunks that pace the PE ramp.
                    bias_src = wt_d[D : D + 1, :]
                    nc.gpsimd.dma_start(
                        out=wbias_bc,
                        in_=bass.AP(
                            tensor=bias_src.tensor,
                            offset=bias_src.offset,
                            ap=[[0, 128]] + list(bias_src.ap[1:]),
                        ).bitcast(F32),
                    )
            for k in range(NK):
                rings[XT_RING[k]].dma_start(
                    out=xt_tiles[k][:, H:BLOC],
                    in_=xt_d[128 * k : 128 * (k + 1), H:BLOC],
                )

            # Full-K accumulation per 128-row batch tile; bufs=4 on each
            # PSUM tag -> 4 b-tiles in flight across all 8 banks.
            for b in range(NB):
                pA = ps_mm.tile([128, NA], F32, tag="pA")
                pB = ps_mm.tile([128, NBK], F32, tag="pB")
                for k in range(NK):
                    xt_ap = xt_tiles[k][:, 128 * b : 128 * (b + 1)]
                    w = wt_tiles[k]
                    nc.tensor.matmul(
                        pA[:], xt_ap, w[:, 0:NA],
                        start=(k == 0), stop=(k == NK - 1),
                    )
                    nc.tensor.matmul(
                        pB[:], xt_ap, w[:, NA:TP],
                        start=(k == 0), stop=(k == NK - 1),
                    )

                # Bias row added on DVE via a partition-broadcast read —
                # keeps the K=1 ones-row matmuls (and their LDWEIGHTS)
                # off the PE stream.
                pre = hazp.tile([128, 2, NA], F32, tag="pre")
                nc.vector.tensor_add(pre[:, 0, :], pA[:], wbias_bc[:, 0:NA])
                nc.vector.tensor_add(pre[:, 1, :], pB[:], wbias_bc[:, NA:TP])

                haz = hazp.tile([128, T], F32, tag="haz")
                base = hazp.tile([128, 1], F32, tag="base")
                nc.scalar.activation(
                    out=haz[:, 0:NA], in_=pre[:, 0, :],
                    func=mybir.ActivationFunctionType.Relu,
                )
                nc.scalar.activation(
                    out=haz[:, NA:T], in_=pre[:, 1, 0:BOFF],
                    func=mybir.ActivationFunctionType.Relu,
                )
                nc.scalar.copy(out=base, in_=pre[:, 1, BOFF : BOFF + 1])

                cum = outp.tile([128, T], F32)
                nc.vector.tensor_tensor_scan(
                    out=cum,
                    data0=haz,
                    data1=zeros,
                    initial=base,
                    op0=mybir.AluOpType.add,
                    op1=mybir.AluOpType.add,
                )
                nc.scalar.dma_start(out=out_d[128 * b : 128 * (b + 1), :], in_=cum)

    nc.compile()
    return nc


_NC_CACHE = None


def kernel(x, W_hazard, b_hazard, W_base, b_base):
    global _NC_CACHE
    if _NC_CACHE is None:
        _NC_CACHE = _build_program()
    nc = _NC_CACHE

    x = np.asarray(x, dtype=np.float32)
    W_cat = np.concatenate(
        [np.asarray(W_hazard, np.float32), np.asarray(W_base, np.float32)], axis=0
    )  # [513, 2048]
    bias_row = np.concatenate(
        [np.asarray(b_hazard, np.float32), np.asarray(b_base, np.float32)]
    )  # [513]
    wt = np.concatenate([W_cat.T, bias_row[None, :]], axis=0)  # [2049, 513]
    wt = np.ascontiguousarray(
        np.concatenate([wt, np.zeros((D + 1, TP - (T + 1)), np.float32)], axis=1)
    )  # [2049, 516]

    in_maps = [
        {
            "xt": np.ascontiguousarray(x[BLOC * i : BLOC * (i + 1)].T),
            "wt": wt,
        }
        for i in range(NCORES)
    ]
    res = run_bass_kernel_spmd(nc, in_maps, list(range(NCORES)))
    return np.concatenate([res.results[i]["out"] for i in range(NCORES)], axis=0)



# revision 3
# speedup vs baseline: 1.5032x; 1.5032x over previous
"""Cumulative-probability head on 8 Trainium2 NeuronCores.

out[b, j] = sum_{i<=j} relu(x @ W_h^T + b_h)[b, i] + (x @ W_base^T + b_base)[b]

Data-parallel: x is sharded along batch (1024 rows per core); the small
weights are replicated. Inputs are cast to fp8 e4m3 on the host (the
2e-2 rel-err budget dwarfs the ~4e-3 this costs) so the matmul runs in
DoubleRow mode: 2 fp8 weights per PE cell, K=256 contracted per pass —
half the PE stream cycles of fp32r, and a quarter of the input DMA
bytes. Per core:

  1. xt4: per-batch-tile blocks [128, 16, 128] fp8, block b holding all
     of K for batch rows 128b..128b+127 (contiguous 256 KB DMA each, so
     b-tile 0's matmuls start ~1.5 us in). wt3: [128, 16, 528] fp8
     (528 = 512 hazard cols + base col + 15 pad, keeping the DoubleRow
     k-pair stride 1056 B, a multiple of 16).
  2. Per b-tile: 8 DoubleRow accumulation steps (k-pairs) into two PSUM
     banks of 264 fp32 cols each; bufs=4 per bank tag -> 4 b-tiles in
     flight across all 8 banks.
  3. Bias row (fp32, never quantized) added on DVE via a
     partition-broadcast read; ReLU on ScalarE (hazard cols only);
     inclusive cumsum along T on DVE tensor_tensor_scan with the base
     hazard as initial state, emitting bf16 to halve output DMA bytes.
"""

import ml_dtypes
import numpy as np

import concourse.bass as bass
import concourse.tile as tile
from concourse import bacc, mybir
from concourse.bass_utils import run_bass_kernel_spmd

B, D, T = 8192, 2048, 512
NCORES = 8
BLOC = B // NCORES            # 1024 rows per core
NB = BLOC // 128              # 8 batch tiles per core
NKS = D // 128                # 16 contraction subtiles
NU = NKS // 2                 # 8 DoubleRow k-pairs
TPW = 528                     # padded output width (512 hazard + base + 15 pad)
NA = TPW // 2                 # cols 0..263 in PSUM bank A
BOFF = T - NA                 # base col offset inside bank B (248)

F32 = mybir.dt.float32
BF16 = mybir.dt.bfloat16
F8 = mybir.dt.float8e4
DR = mybir.MatmulPerfMode.DoubleRow
NP_F8 = ml_dtypes.float8_e4m3
NP_BF16 = ml_dtypes.bfloat16


def _build_program():
    nc = bacc.Bacc("TRN2", target_bir_lowering=False, debug=False)

    xt_d = nc.dram_tensor("xt", [NB, 128, NKS * 128], F8, kind="ExternalInput")
    wt_d = nc.dram_tensor("wt", [128, NKS, TPW], F8, kind="ExternalInput")
    bias_d = nc.dram_tensor("bias", [1, TPW], F32, kind="ExternalInput")
    out_d = nc.dram_tensor("out", [BLOC, T], BF16, kind="ExternalOutput")

    with tile.TileContext(nc) as tc:
        with (
            tc.tile_pool(name="consts", bufs=1) as consts,
            tc.tile_pool(name="wt", bufs=1) as wtp,
            tc.tile_pool(name="xt", bufs=1) as xtp,
            tc.tile_pool(name="haz", bufs=4) as hazp,
            tc.tile_pool(name="outp", bufs=4) as outp,
            tc.tile_pool(name="ps_mm", bufs=4, space="PSUM") as ps_mm,
        ):
            zeros = consts.tile([128, T], F32)
            nc.vector.memset(zeros, 0.0)

            rings = [nc.sync, nc.scalar, nc.gpsimd]

            # Weights as 8 k-pair chunks (1056 B/partition each,
            # contiguous); first chunks on the fast HWDGE rings since
            # they gate the PE ramp.
            wt_sb = wtp.tile([128, NKS, TPW], F8, tag="wt")
            W_RING = [0, 1, 0, 1, 2, 0, 1, 2]
            for u in range(NU):
                rings[W_RING[u]].dma_start(
                    out=wt_sb[:, 2 * u : 2 * u + 2, :],
                    in_=wt_d[:, 2 * u : 2 * u + 2, :],
                )

            # Bias row broadcast to all 128 partitions with a
            # partition-stride-0 DMA read (engines can't read stride-0
            # partition APs, but DMA can).
            wbias_bc = wtp.tile([128, TPW], F32, tag="wbias")
            bias_src = bias_d[0:1, :]
            nc.gpsimd.dma_start(
                out=wbias_bc,
                in_=bass.AP(
                    tensor=bias_src.tensor,
                    offset=bias_src.offset,
                    ap=[[0, 128]] + list(bias_src.ap[1:]),
                ),
            )

            # x blocks, batch-tile-major: block b = all of K for batch
            # rows 128b..128b+127, one contiguous 256 KB transfer
            # (block 0 split in half so b=0's first matmuls start
            # sooner).
            X_RING = [1, 0, 1, 2, 0, 1, 2, 0]
            xb = []
            for b in range(NB):
                xt_sb = xtp.tile([128, NKS, 128], F8, tag=f"xb{b}")
                if b == 0:
                    nc.sync.dma_start(
                        out=xt_sb[:, 0:8, :], in_=xt_d[0, :, 0 : 8 * 128]
                    )
                    nc.scalar.dma_start(
                        out=xt_sb[:, 8:NKS, :], in_=xt_d[0, :, 8 * 128 :]
                    )
                else:
                    rings[X_RING[b]].dma_start(
                        out=xt_sb[:], in_=xt_d[b, :, :]
                    )
                xb.append(xt_sb)

            for b in range(NB):
                pA = ps_mm.tile([128, NA], F32, tag="pA")
                pB = ps_mm.tile([128, NA], F32, tag="pB")
                for u in range(NU):
                    lhsT = xb[b][:, 2 * u : 2 * u + 2, :]
                    w = wt_sb[:, 2 * u : 2 * u + 2, :]
                    nc.tensor.matmul(
                        pA[:], lhsT, w[:, :, 0:NA],
                        start=(u == 0), stop=(u == NU - 1), perf_mode=DR,
                    )
                    nc.tensor.matmul(
                        pB[:], lhsT, w[:, :, NA:TPW],
                        start=(u == 0), stop=(u == NU - 1), perf_mode=DR,
                    )

                # Bias row added on DVE via the partition-broadcast
                # tile — keeps K=1 ones-row matmuls off the PE stream.
                pre = hazp.tile([128, 2, NA], F32, tag="pre")
                nc.vector.tensor_add(pre[:, 0, :], pA[:], wbias_bc[:, 0:NA])
                nc.vector.tensor_add(pre[:, 1, :], pB[:], wbias_bc[:, NA:TPW])

                haz = hazp.tile([128, T], F32, tag="haz")
                base = hazp.tile([128, 1], F32, tag="base")
                nc.scalar.activation(
                    out=haz[:, 0:NA], in_=pre[:, 0, :],
                    func=mybir.ActivationFunctionType.Relu,
                )
                nc.scalar.activation(
                    out=haz[:, NA:T], in_=pre[:, 1, 0:BOFF],
                    func=mybir.ActivationFunctionType.Relu,
                )
                nc.scalar.copy(out=base, in_=pre[:, 1, BOFF : BOFF + 1])

                cum = outp.tile([128, T], BF16)
                nc.vector.tensor_tensor_scan(
                    out=cum,
                    data0=haz,
                    data1=zeros,
                    initial=base,
                    op0=mybir.AluOpType.add,
                    op1=mybir.AluOpType.add,
                )
                nc.scalar.dma_start(out=out_d[128 * b : 128 * (b + 1), :], in_=cum)

    nc.compile()
    return nc


def _prep_in_maps(x, W_hazard, b_hazard, W_base, b_base):
    x = np.asarray(x, dtype=np.float32)
    W_aug = np.zeros((TPW, D), np.float32)
    W_aug[0:T] = np.asarray(W_hazard, np.float32)
    W_aug[T] = np.asarray(W_base, np.float32)[0]
    wt3 = np.ascontiguousarray(
        W_aug.T.astype(NP_F8).reshape(NKS, 128, TPW).transpose(1, 0, 2)
    )  # [128, 16, 528] fp8
    bias_row = np.zeros((1, TPW), np.float32)
    bias_row[0, 0:T] = np.asarray(b_hazard, np.float32)
    bias_row[0, T] = np.asarray(b_base, np.float32)[0]

    x8 = x.astype(NP_F8)
    in_maps = []
    for i in range(NCORES):
        shard = x8[BLOC * i : BLOC * (i + 1)]  # [1024, 2048]
        # xt4[b, p, j*128+c] = shard[128b+c, 128j+p]
        xt4 = np.ascontiguousarray(
            shard.reshape(NB, 128, NKS, 128)  # [b, c, j, p]
            .transpose(0, 3, 2, 1)            # [b, p, j, c]
            .reshape(NB, 128, NKS * 128)
        )
        in_maps.append({"xt": xt4, "wt": wt3, "bias": bias_row})
    return in_maps


_NC_CACHE = None


def kernel(x, W_hazard, b_hazard, W_base, b_base):
    global _NC_CACHE
    if _NC_CACHE is None:
        _NC_CACHE = _build_program()
    nc = _NC_CACHE

    in_maps = _prep_in_maps(x, W_hazard, b_hazard, W_base, b_base)
    res = run_bass_kernel_spmd(nc, in_maps, list(range(NCORES)))
    return np.concatenate(
        [res.results[i]["out"].astype(np.float32) for i in range(NCORES)], axis=0
    )


# revision 5
# speedup vs baseline: 1.6174x; 1.0760x over previous
"""Cumulative-probability head on 8 Trainium2 NeuronCores.

out[b, j] = sum_{i<=j} relu(x @ W_h^T + b_h)[b, i] + (x @ W_base^T + b_base)[b]

Data-parallel: x is sharded along batch (1024 rows per core); the small
weights are replicated. Inputs are cast to fp8 e4m3 on the host (the
2e-2 rel-err budget dwarfs the ~4e-3 this costs) so the matmul runs in
DoubleRow mode: 2 fp8 weights per PE cell, K=256 contracted per pass —
half the PE stream cycles of fp32r, and a quarter of the input DMA
bytes. Per core:

  1. xt4: per-batch-tile blocks [128, 16, 128] fp8, block b holding all
     of K for batch rows 128b..128b+127 (contiguous 256 KB DMA each).
     wt3: [128, 16, 528] fp8 (528 = 512 hazard cols + base col + 15
     pad, keeping the DoubleRow k-pair stride 1056 B, a multiple of
     16). DMA FIFO order per ring is need-order: w0, x-block 0, then
     the rest interleaved — a ring is in-order, so anything queued
     ahead of block 0 delays the first matmul.
  2. Per b-tile: 8 DoubleRow accumulation steps (k-pairs) into one
     two-bank PSUM tile [128, 2, 512] (cols 0..263 of each bank);
     bufs=4 -> 4 b-tiles in flight across all 8 banks.
  3. Epilogue is one op per engine per b-tile: a single DVE add of the
     partition-broadcast bias row over a strided 2-bank PSUM read, a
     single ScalarE ReLU over the 512 hazard cols (bf16 out), and the
     DVE cumsum scan (tensor_tensor_scan) seeded with the base-hazard
     column read straight out of the pre-activation tile. Output DMA
     rides the Sync ring (ScalarE is busy with ReLUs), in bf16 to
     halve write bytes.
"""

import ml_dtypes
import numpy as np

import concourse.bass as bass
import concourse.tile as tile
from concourse import bacc, mybir
from concourse.bass_utils import run_bass_kernel_spmd

B, D, T = 8192, 2048, 512
NCORES = 8
BLOC = B // NCORES            # 1024 rows per core
NB = BLOC // 128              # 8 batch tiles per core
NKS = D // 128                # 16 contraction subtiles
NU = NKS // 2                 # 8 DoubleRow k-pairs
TPW = 528                     # padded output width (512 hazard + base + 15 pad)
NA = TPW // 2                 # 264 cols per PSUM bank
BCOL = T                      # flat index of the base col in pre (512)

F32 = mybir.dt.float32
BF16 = mybir.dt.bfloat16
F8 = mybir.dt.float8e4
DR = mybir.MatmulPerfMode.DoubleRow
NP_F8 = ml_dtypes.float8_e4m3
NP_BF16 = ml_dtypes.bfloat16


def _flat(ap, n, offset_elems=0):
    """View the free dims of a contiguous 128-partition AP as [128, n]."""
    return bass.AP(
        tensor=ap.tensor,
        offset=ap.offset + offset_elems,
        ap=[list(ap.ap[0]), [1, n]],
    )


def _build_program():
    nc = bacc.Bacc("TRN2", target_bir_lowering=False, debug=False)

    xt_d = nc.dram_tensor("xt", [NB, 128, NKS * 128], F8, kind="ExternalInput")
    wt_d = nc.dram_tensor("wt", [128, NKS, TPW], F8, kind="ExternalInput")
    bias_d = nc.dram_tensor("bias", [1, TPW], F32, kind="ExternalInput")
    out_d = nc.dram_tensor("out", [BLOC, T], BF16, kind="ExternalOutput")

    with tile.TileContext(nc) as tc:
        with (
            tc.tile_pool(name="consts", bufs=1) as consts,
            tc.tile_pool(name="wt", bufs=1) as wtp,
            tc.tile_pool(name="xt", bufs=1) as xtp,
            tc.tile_pool(name="haz", bufs=4) as hazp,
            tc.tile_pool(name="outp", bufs=4) as outp,
            tc.tile_pool(name="ps_mm", bufs=4, space="PSUM") as ps_mm,
        ):
            zeros = consts.tile([128, T], BF16)
            nc.vector.memset(zeros, 0.0)

            wt_sb = wtp.tile([128, NKS, TPW], F8, tag="wt")
            xb = [
                xtp.tile([128, NKS, 128], F8, tag=f"xb{b}", name=f"xb{b}")
                for b in range(NB)
            ]

            def load_w(ring, u):
                ring.dma_start(
                    out=wt_sb[:, 2 * u : 2 * u + 2, :],
                    in_=wt_d[:, 2 * u : 2 * u + 2, :],
                )

            def load_x(ring, b):
                ring.dma_start(out=xb[b][:], in_=xt_d[b, :, :])

            # Need-ordered FIFO per ring: w0/w1 then both halves of x
            # block 0 gate the first matmuls; later w chunks and x
            # blocks interleave so each ring's arrival order tracks the
            # PE's consumption order.
            load_w(nc.sync, 0)
            load_w(nc.scalar, 1)
            nc.sync.dma_start(out=xb[0][:, 0:8, :], in_=xt_d[0, :, 0 : 8 * 128])
            nc.scalar.dma_start(out=xb[0][:, 8:NKS, :], in_=xt_d[0, :, 8 * 128 :])
            # Bias row broadcast to all 128 partitions with a
            # partition-stride-0 DMA read (engines can't read stride-0
            # partition APs, but DMA can).
            wbias_bc = wtp.tile([128, TPW], F32, tag="wbias")
            bias_src = bias_d[0:1, :]
            nc.gpsimd.dma_start(
                out=wbias_bc,
                in_=bass.AP(
                    tensor=bias_src.tensor,
                    offset=bias_src.offset,
                    ap=[[0, 128]] + list(bias_src.ap[1:]),
                ),
            )
            load_w(nc.sync, 2)
            load_w(nc.scalar, 3)
            load_x(nc.sync, 1)
            load_x(nc.scalar, 2)
            load_x(nc.gpsimd, 3)
            load_w(nc.sync, 4)
            load_w(nc.scalar, 5)
            load_x(nc.sync, 4)
            load_x(nc.scalar, 5)
            load_w(nc.sync, 6)
            load_w(nc.scalar, 7)
            load_x(nc.sync, 6)
            load_x(nc.scalar, 7)

            for b in range(NB):
                # Two accumulation regions in one 2-bank tile so the
                # bias add is a single strided DVE op.
                pAB = ps_mm.tile([128, 2, 512], F32, tag="pAB")
                for u in range(NU):
                    lhsT = xb[b][:, 2 * u : 2 * u + 2, :]
                    w = wt_sb[:, 2 * u : 2 * u + 2, :]
                    nc.tensor.matmul(
                        pAB[:, 0, 0:NA], lhsT, w[:, :, 0:NA],
                        start=(u == 0), stop=(u == NU - 1), perf_mode=DR,
                    )
                    nc.tensor.matmul(
                        pAB[:, 1, 0:NA], lhsT, w[:, :, NA:TPW],
                        start=(u == 0), stop=(u == NU - 1), perf_mode=DR,
                    )

                # pre flat layout: cols 0..511 hazards, 512 base, rest pad.
                pre = hazp.tile([128, 2, NA], F32, tag="pre")
                nc.vector.tensor_add(
                    pre[:],
                    pAB[:, :, 0:NA],
                    bass.AP(
                        tensor=wbias_bc.tensor,
                        offset=wbias_bc.offset,
                        ap=[list(wbias_bc.ap[0]), [NA, 2], [1, NA]],
                    ),
                )
                haz = hazp.tile([128, T], BF16, tag="haz")
                nc.scalar.activation(
                    out=haz, in_=_flat(pre, T),
                    func=mybir.ActivationFunctionType.Relu,
                )
                cum = outp.tile([128, T], BF16)
                nc.vector.tensor_tensor_scan(
                    out=cum,
                    data0=haz,
                    data1=zeros,
                    initial=_flat(pre, 1, BCOL),
                    op0=mybir.AluOpType.add,
                    op1=mybir.AluOpType.add,
                )
                nc.sync.dma_start(out=out_d[128 * b : 128 * (b + 1), :], in_=cum)

    nc.compile()
    return nc


def _prep_in_maps(x, W_hazard, b_hazard, W_base, b_base):
    x = np.asarray(x, dtype=np.float32)
    W_aug = np.zeros((TPW, D), np.float32)
    W_aug[0:T] = np.asarray(W_hazard, np.float32)
    W_aug[T] = np.asarray(W_base, np.float32)[0]
    wt3 = np.ascontiguousarray(
        W_aug.T.astype(NP_F8).reshape(NKS, 128, TPW).transpose(1, 0, 2)
    )  # [128, 16, 528] fp8
    bias_row = np.zeros((1, TPW), np.float32)
    bias_row[0, 0:T] = np.asarray(b_hazard, np.float32)
    bias_row[0, T] = np.asarray(b_base, np.float32)[0]

    x8 = x.astype(NP_F8)
    in_maps = []
    for i in range(NCORES):
        shard = x8[BLOC * i : BLOC * (i + 1)]  # [1024, 2048]
        # xt4[b, p, j*128+c] = shard[128b+c, 128j+p]
        xt4 = np.ascontiguousarray(
            shard.reshape(NB, 128, NKS, 128)  # [b, c, j, p]
            .transpose(0, 3, 2, 1)            # [b, p, j, c]
            .reshape(NB, 128, NKS * 128)
        )
        in_maps.append({"xt": xt4, "wt": wt3, "bias": bias_row})
    return in_maps


_NC_CACHE = None


def kernel(x, W_hazard, b_hazard, W_base, b_base):
    global _NC_CACHE
    if _NC_CACHE is None:
        _NC_CACHE = _build_program()
    nc = _NC_CACHE

    in_maps = _prep_in_maps(x, W_hazard, b_hazard, W_base, b_base)
    res = run_bass_kernel_spmd(nc, in_maps, list(range(NCORES)))
    return np.concatenate(
        [res.results[i]["out"].astype(np.float32) for i in range(NCORES)], axis=0
    )


# revision 7
# speedup vs baseline: 1.7115x; 1.0582x over previous
"""Cumulative-probability head on 8 Trainium2 NeuronCores.

out[b, j] = sum_{i<=j} relu(x @ W_h^T + b_h)[b, i] + (x @ W_base^T + b_base)[b]

Data-parallel: x is sharded along batch (1024 rows per core); the small
weights are replicated. Inputs are cast to fp8 e4m3 on the host (the
2e-2 rel-err budget dwarfs the ~4e-3 this costs) so the matmul runs in
DoubleRow mode: 2 fp8 weights per PE cell, K=256 contracted per pass —
half the PE stream cycles of fp32r, and a quarter of the input DMA
bytes. Per core:

  1. xt4: per-batch-tile blocks [128, 17, 128] fp8 — block b holds all
     of K for batch rows 128b..128b+127 plus a scaled-ones subtile
     (0.0625) that folds the bias add into the matmul. wt3:
     [128, 18, 528] fp8 (528 = 512 hazard cols + base col + 15 pad,
     keeping the DoubleRow k-pair stride 1056 B a multiple of 16);
     subtile 16 is zeros, 17 carries 16*bias in partition 0 only, so
     k-pair 8 (lhsT subtiles 15:17 against rhs subtiles 16:18)
     contributes exactly the bias row. DMA FIFO order per ring is
     need-order: w pair 0, x block 0, then the rest interleaved.
  2. ~36 back-to-back dummy matmuls on a zeroed tile run while the
     first inputs stream in: the PE HAM clock-gate needs ~3.4 us of
     sustained busy to lift the 1.2 GHz cold throttle, and the dummies
     burn that window so the real matmuls run at 2.4 GHz from b-tile 0
     (which also unblocks the DVE epilogue ~6 us earlier).
  3. Per b-tile: 9 DoubleRow accumulation steps into one two-bank PSUM
     tile [128, 2, 512] (cols 0..263 of each bank); bufs=4 -> 4
     b-tiles in flight across all 8 banks.
  4. Epilogue per b-tile: one ScalarE ReLU over a strided 2-bank PSUM
     read (bf16 out), then the DVE cumsum (tensor_tensor_scan) seeded
     with the base-hazard column read straight from PSUM. Output DMA
     in bf16, alternating Sync/GpSimd rings (ScalarE stays on ReLUs).
"""

import ml_dtypes
import numpy as np

import concourse.bass as bass
import concourse.tile as tile
from concourse import bacc, mybir
from concourse.bass_utils import run_bass_kernel_spmd

B, D, T = 8192, 2048, 512
NCORES = 8
BLOC = B // NCORES            # 1024 rows per core
NB = BLOC // 128              # 8 batch tiles per core
NKS = D // 128                # 16 data contraction subtiles
NXS = NKS + 1                 # +1 scaled-ones subtile in x
NWS = NKS + 2                 # +zeros +bias subtiles in w
NU = NKS // 2 + 1             # 9 DoubleRow k-pairs (8 data + bias)
TPW = 528                     # padded output width (512 hazard + base + 15 pad)
NA = TPW // 2                 # 264 cols per PSUM bank
NWARM = 36                    # HAM warmup matmuls
ONES = 0.0625                 # exact in e4m3; bias row is scaled by 1/ONES

F32 = mybir.dt.float32
BF16 = mybir.dt.bfloat16
F8 = mybir.dt.float8e4
DR = mybir.MatmulPerfMode.DoubleRow
NP_F8 = ml_dtypes.float8_e4m3


def _build_program():
    nc = bacc.Bacc("TRN2", target_bir_lowering=False, debug=False)

    xt_d = nc.dram_tensor("xt", [NB, 128, NXS * 128], F8, kind="ExternalInput")
    wt_d = nc.dram_tensor("wt", [128, NWS, TPW], F8, kind="ExternalInput")
    out_d = nc.dram_tensor("out", [BLOC, T], BF16, kind="ExternalOutput")

    with tile.TileContext(nc) as tc:
        with (
            tc.tile_pool(name="consts", bufs=1) as consts,
            tc.tile_pool(name="wt", bufs=1) as wtp,
            tc.tile_pool(name="xt", bufs=1) as xtp,
            tc.tile_pool(name="haz", bufs=4) as hazp,
            tc.tile_pool(name="outp", bufs=4) as outp,
            tc.tile_pool(name="ps_mm", bufs=4, space="PSUM") as ps_mm,
        ):
            zeros = consts.tile([128, T], BF16)
            nc.vector.memset(zeros, 0.0)
            warm_f8 = consts.tile([128, 128], F8)
            nc.vector.memset(warm_f8, 0.0)

            wt_sb = wtp.tile([128, NWS, TPW], F8, tag="wt")
            xb = [
                xtp.tile([128, NXS, 128], F8, tag=f"xb{b}", name=f"xb{b}")
                for b in range(NB)
            ]

            def load_w(ring, u):
                ring.dma_start(
                    out=wt_sb[:, 2 * u : 2 * u + 2, :],
                    in_=wt_d[:, 2 * u : 2 * u + 2, :],
                )

            def load_x(ring, b):
                ring.dma_start(out=xb[b][:], in_=xt_d[b, :, :])

            # Need-ordered FIFO per ring: w pair 0 and both halves of x
            # block 0 gate the first matmuls; later w chunks and x
            # blocks interleave to track the PE's consumption order.
            load_w(nc.sync, 0)
            load_w(nc.scalar, 1)
            nc.sync.dma_start(out=xb[0][:, 0:8, :], in_=xt_d[0, :, 0 : 8 * 128])
            nc.scalar.dma_start(out=xb[0][:, 8:NXS, :], in_=xt_d[0, :, 8 * 128 :])
            load_w(nc.gpsimd, 8)
            load_x(nc.gpsimd, 3)
            load_w(nc.sync, 2)
            load_w(nc.scalar, 3)
            load_x(nc.sync, 1)
            load_x(nc.scalar, 2)
            load_w(nc.sync, 4)
            load_w(nc.scalar, 5)
            load_x(nc.sync, 4)
            load_x(nc.scalar, 5)
            load_w(nc.sync, 6)
            load_w(nc.scalar, 7)
            load_x(nc.sync, 6)
            load_x(nc.scalar, 7)

            # HAM warmup: the dummies depend only on the memset tile, so
            # they run during the input-DMA window and lift the PE clock
            # gate to 8/8 before the first real matmul.
            ps_warm = ps_mm.tile([128, 2, 512], F32, tag="pAB", name="ps_warm")
            for i in range(NWARM):
                nc.tensor.matmul(
                    ps_warm[:, 0, 0:128], warm_f8, warm_f8,
                    start=True, stop=True, skip_group_check=True,
                )

            for b in range(NB):
                # Two accumulation regions in one 2-bank tile so the
                # ReLU is a single strided read over both banks.
                pAB = (
                    ps_warm if b == 0
                    else ps_mm.tile([128, 2, 512], F32, tag="pAB", name="pAB")
                )
                for u in range(NU):
                    if u < NU - 1:
                        lhsT = xb[b][:, 2 * u : 2 * u + 2, :]
                        w = wt_sb[:, 2 * u : 2 * u + 2, :]
                    else:
                        # Bias pair: x subtiles 15:17 (data, ones)
                        # against w subtiles 16:18 (zeros, 16*bias) —
                        # the data subtile is annihilated by the zero
                        # weights, the ones row injects the bias.
                        lhsT = xb[b][:, NKS - 1 : NXS, :]
                        w = wt_sb[:, NKS : NWS, :]
                    nc.tensor.matmul(
                        pAB[:, 0, 0:NA], lhsT, w[:, :, 0:NA],
                        start=(u == 0), stop=(u == NU - 1), perf_mode=DR,
                        skip_group_check=(b == 0),
                    )
                    nc.tensor.matmul(
                        pAB[:, 1, 0:NA], lhsT, w[:, :, NA:TPW],
                        start=(u == 0), stop=(u == NU - 1), perf_mode=DR,
                        skip_group_check=(b == 0),
                    )

                # PSUM flat layout: cols 0..511 hazards, 512 base; the
                # relu'd base/pad cols 512..527 are junk the scan skips.
                haz = hazp.tile([128, TPW], BF16, tag="haz")
                nc.scalar.activation(
                    out=haz, in_=pAB[:, :, 0:NA],
                    func=mybir.ActivationFunctionType.Relu,
                )
                cum = outp.tile([128, T], BF16)
                nc.vector.tensor_tensor_scan(
                    out=cum,
                    data0=haz[:, 0:T],
                    data1=zeros,
                    initial=pAB[:, 1, T - NA : T - NA + 1],
                    op0=mybir.AluOpType.add,
                    op1=mybir.AluOpType.add,
                )
                ring = nc.sync if b % 2 == 0 else nc.gpsimd
                ring.dma_start(out=out_d[128 * b : 128 * (b + 1), :], in_=cum)

    nc.compile()
    return nc


def _prep_in_maps(x, W_hazard, b_hazard, W_base, b_base):
    x = np.asarray(x, dtype=np.float32)
    W_aug = np.zeros((TPW, D), np.float32)
    W_aug[0:T] = np.asarray(W_hazard, np.float32)
    W_aug[T] = np.asarray(W_base, np.float32)[0]
    wt3 = np.zeros((128, NWS, TPW), NP_F8)
    wt3[:, 0:NKS, :] = (
        W_aug.T.astype(NP_F8).reshape(NKS, 128, TPW).transpose(1, 0, 2)
    )
    bias_row = np.zeros(TPW, np.float32)
    bias_row[0:T] = np.asarray(b_hazard, np.float32)
    bias_row[T] = np.asarray(b_base, np.float32)[0]
    wt3[0, NKS + 1, :] = (bias_row / ONES).astype(NP_F8)
    wt3 = np.ascontiguousarray(wt3)

    x8 = x.astype(NP_F8)
    in_maps = []
    for i in range(NCORES):
        shard = x8[BLOC * i : BLOC * (i + 1)]  # [1024, 2048]
        # xt4[b, p, j*128+c] = shard[128b+c, 128j+p]; subtile 16 = ONES
        xt4 = np.full((NB, 128, NXS * 128), ONES, NP_F8)
        xt4[:, :, 0 : NKS * 128] = (
            shard.reshape(NB, 128, NKS, 128)  # [b, c, j, p]
            .transpose(0, 3, 2, 1)            # [b, p, j, c]
            .reshape(NB, 128, NKS * 128)
        )
        in_maps.append({"xt": np.ascontiguousarray(xt4), "wt": wt3})
    return in_maps


_NC_CACHE = None


def kernel(x, W_hazard, b_hazard, W_base, b_base):
    global _NC_CACHE
    if _NC_CACHE is None:
        _NC_CACHE = _build_program()
    nc = _NC_CACHE

    in_maps = _prep_in_maps(x, W_hazard, b_hazard, W_base, b_base)
    res = run_bass_kernel_spmd(nc, in_maps, list(range(NCORES)))
    return np.concatenate(
        [res.results[i]["out"].astype(np.float32) for i in range(NCORES)], axis=0
    )
